# revision 39
# baseline (speedup 1.0000x reference)
"""Trainium2 Bass kernel for a 2-layer dense GCN (NodeEncoder).

    out = adj @ relu(adj @ (x@W1) + b1) @ W2 + b2
    N=16384, F_IN=512, HID=1024, OUT=256, adj dense [N, N] fp32.

Key algebraic optimization vs the straightforward lowering: layer 1 is
computed as (adj @ x) @ W1 instead of adj @ (x @ W1).  The adj
contraction then runs against F_IN=512 columns instead of HID=1024,
halving the dominant matmul's FLOPs (275 vs 550 GFLOP), and since x is
replicated on every core the layer-1 AllGather disappears entirely.

Sharding: adj row-partitioned across 8 NeuronCores (2048 rows/core).
Per core (all matmuls bf16 with fp32 PSUM accumulation):

  phase A:  zT_c   = (adj_c @ x)^T          [512, 2048]   (lhsT = x
            blocks stationary, rhs = adjT_c streaming; out is zT)
  phase H:  hT_c   = relu(z_c @ W1 + b1)^T  [1024, 2048]  (lhsT = W1
            blocks, rhs = zT tiles; bias per-partition in ACT relu)
  phase S:  s2_c   = h_c @ W2               [2048, 256]   (lhsT = hT
            blocks, rhs = W2)
  AG:       s2     = AllGather(s2_c)        [16384, 256]  (in quarters,
            fired as soon as each quarter of s2_c is ready)
  phase D:  out2T_c = (adj_c @ s2)^T + b2   [256, 2048]   (lhsT = s2
            tiles, rhs = adjT_c streaming; b2 via ACT Identity)

Phases A/H/S are split in two m-chunks (1024 adj columns each) so the
first two AG quarters fire halfway through phase A and the gather
overlaps compute; phase D consumes k-blocks in gather-arrival order.
"""

import numpy as np
import ml_dtypes

import concourse.bass as bass
import concourse.mybir as mybir
import concourse.tile as tile
from concourse.bass_utils import run_bass_kernel_spmd
from concourse.tile_sem_assignment import N_PROCS
from concourse.vector_clock import ScopedClock, VectorClock

# ---------------------------------------------------------------------------
# Workaround: the walrus build in this container caps the number of sync-wait
# commands on a Drain instruction; Tile's kernel-tail drain aggregates one
# wait per logical processor and exceeds it.  Split the tail drain into a
# chain of single-wait drains on the same (SP) queue — semantically identical.
# ---------------------------------------------------------------------------


def _drain_and_barrier_split(self, tick_clock, wait_clock):
    gc = tick_clock.global_clock
    for p in range(N_PROCS):
        partial = VectorClock([gc[q] if q == p else 0 for q in range(N_PROCS)])
        d = self.nc.sync.drain()
        wait_clock.add_sem_waits(d.ins, ScopedClock({None: partial}))
    self.nc.sync.drain()

    self.nc.all_engine_barrier()
    assert self.sems is not None
    popped = self.nc._tile_sem_poison_stack.pop()
    assert popped is self._sem_poison
    self.nc.clear_and_free_semaphores(list(self.sems.allocated().values()))
    self.nc.all_engine_barrier()


tile.TileContext._drain_and_barrier = _drain_and_barrier_split

# The same walrus cap applies to every instruction kind: at most ONE sync
# wait command per instruction (probed empirically — a 2-wait TensorCopy is
# rejected).  Post-pass: hoist excess sem-waits onto no-ops inserted just
# before the instruction on the same engine queue — per-engine program order
# makes this semantically identical.
_MAX_WAITS = 1


def _split_excess_waits(nc):
    ctr = 0
    for f in nc.m.functions:
        for bb in f.blocks:
            out = []
            changed = False
            for inst in bb.instructions:
                si = inst.sync_info
                waits = list(si.on_wait) if si is not None and si.on_wait else []
                if len(waits) > _MAX_WAITS:
                    changed = True
                    keep, excess = waits[: _MAX_WAITS], waits[_MAX_WAITS :]
                    for i in range(0, len(excess), _MAX_WAITS):
                        ctr += 1
                        nop = mybir.InstNoOp(name=f"I-waitnop-{ctr}")
                        nop.engine = inst.engine
                        nop.sync_info = mybir.SyncInfo(
                            on_wait=excess[i : i + _MAX_WAITS], on_update=[]
                        )
                        out.append(nop)
                    si.on_wait = keep
                out.append(inst)
            if changed:
                bb.instructions = out
    return ctr


def _elide_redundant_ldweights(nc):
    """Delete an InstLdweights that reloads the exact weights AP loaded by
    the previous (surviving) InstLdweights when only plain matmuls / no-ops
    sit between them in the scheduled stream.  The PE array keeps the
    stationary operand across matmuls, so the reload is pure overhead
    (walrus emits one LDWEIGHTS per MATMUL and its ldw-opt pass is
    incompatible with pre-split LDW+MM).  Only sync-free LDWs are removed,
    so semaphore bookkeeping is unchanged."""
    n_elided = 0
    for f in nc.m.functions:
        for bb in f.blocks:
            out = []
            last_w = None  # weights-AP repr of last surviving LDW, if run intact
            changed = False
            for inst in bb.instructions:
                nm = type(inst).__name__
                if nm == "InstLdweights":
                    si = inst.sync_info
                    clean = not (si and (si.on_wait or si.on_update))
                    w = repr(inst.ins[0])
                    if clean and last_w == w:
                        n_elided += 1
                        changed = True
                        continue  # drop the reload
                    last_w = w if clean else None
                elif nm == "InstMatmult":
                    if getattr(inst, "is_transpose", False):
                        last_w = None
                elif nm == "InstNoOp":
                    pass
                else:
                    last_w = None
                out.append(inst)
            if changed:
                bb.instructions = out
    return n_elided


NCORES = 8
N = 16384
SH = N // NCORES  # 2048 adj rows per core
F = 512
HID = 1024
OUT = 256

BF16 = mybir.dt.bfloat16
F32 = mybir.dt.float32
FP8 = mybir.dt.float8e4
ADJ_SCALE = float(N)  # adj pre-scaled into fp8 range; 1/N folded into W1
S2_SCALE = 1024.0  # s2 pre-scaled into fp8 range; undone at phase D evac

_built = None


def build():
    """Build the per-core Bass program (identical on all cores)."""
    nc = bass.Bass()

    # All big inputs are host-prepped into partition-major tiled layouts so
    # every DMA reads long contiguous per-partition runs (8-64 KiB):
    #   adjD[p, (k4 kk m)] = adjT[k4*512+kk*128+p, m]  (phases A and D)
    #   xP  [p, (kb f)]    = x[kb*128+p, f]            (replicated)
    adjD = nc.declare_dram_parameter("adjD", [128, 32 * 4 * SH], FP8, isOutput=False)
    xP = nc.declare_dram_parameter("xP", [128, (N // 128) * F], FP8, isOutput=False)
    w1 = nc.declare_dram_parameter("w1", [F, HID], BF16, isOutput=False)
    w2 = nc.declare_dram_parameter("w2", [HID, OUT], BF16, isOutput=False)
    b1T = nc.declare_dram_parameter("b1T", [128, HID // 128], F32, isOutput=False)
    b2T = nc.declare_dram_parameter("b2T", [128, OUT // 128], F32, isOutput=False)
    # rank-1 correction operands for the fp8 phase A (see _prep_inputs):
    #   pre1 += vneg^T . rrow   cancels the coherent x-quantization error
    vneg = nc.declare_dram_parameter("vneg", [1, HID], BF16, isOutput=False)
    rrow = nc.declare_dram_parameter("rrow", [1, SH], BF16, isOutput=False)
    out2T = nc.declare_dram_parameter("out2T", [OUT, SH], F32, isOutput=True)

    rg = [list(range(NCORES))]

    def allgather(inp, outp):
        return nc.gpsimd.collective_compute(
            "AllGather",
            mybir.AluOpType.bypass,
            replica_groups=rg,
            ins=[inp.opt()],
            outs=[outp.opt()],
        )

    with tile.TileContext(nc) as tc:
        with (
            tc.tile_pool(name="const", bufs=1) as constp,
            tc.tile_pool(name="psum", bufs=8, space="PSUM") as psum,
            tc.tile_pool(name="dram", bufs=1, space="DRAM") as dram,
        ):
            # ---- constants (ACT HWDGE ring; adj streams ride the SP ring).
            # Declared here, but the DMAs are issued AFTER the first x tiles
            # below: phase A's first matmul gates on x tile 0, while the
            # weights aren't read until phase H ~270us in.
            w1t = constp.tile([128, F // 128, HID], BF16)
            w2t = constp.tile([128, HID // 128, OUT], BF16)
            b1t = constp.tile([128, HID // 128], F32)
            b2t = constp.tile([128, OUT // 128], F32)
            vnt = constp.tile([1, HID], BF16)
            rrt = constp.tile([1, SH], BF16)

            # AG buffers partition-major: rank contribution [128, skk*256+n]
            # with s2 row skk*128+p; gathered output stacks ranks on dim 0.
            ag_in = [dram.tile([128, 4 * OUT], FP8, name=f"agi{q}") for q in range(4)]
            ag_out = [
                dram.tile([128 * 8, 4 * OUT], FP8, addr_space="Shared", name=f"ago{q}")
                for q in range(4)
            ]

            xsrc = xP[:].rearrange("p (kb f) -> p kb f", f=F)

            with (
                tc.tile_pool(name="xp", bufs=1) as xp,
                tc.tile_pool(name="zt", bufs=16) as ztp,
                tc.tile_pool(name="ht", bufs=16) as htp,
                tc.tile_pool(name="adjA", bufs=6) as adjp,
                tc.tile_pool(name="small", bufs=4) as smallp,
            ):
                xts = []  # 16 tiles of 8 k-blocks each
                zt = {}
                ht = {}
                # phase A streams the SAME full-width k4 tiles as phase D
                # (shared adjD param).  Two passes over f-halves, so each
                # stationary x block feeds 4 matmuls (m 0..2047), halving
                # the (serializing) DoubleRow LDWEIGHTS count.
                a_src = adjD[:].rearrange("p (k4 kk m) -> p k4 kk m", k4=32, kk=4)
                for fh in range(2):
                    ps = [
                        psum.tile([128, 512], F32, tag="ps", name=f"psA{fh}{i}")
                        for i in range(8)
                    ]
                    for k4 in range(32):
                        if fh == 0 and k4 % 2 == 0:
                            i = k4 // 2
                            t = xp.tile([128, 8, F], FP8, name=f"xt{i}")
                            nc.scalar.dma_start(t[:], xsrc[:, i * 8 : (i + 1) * 8])
                            xts.append(t)
                            if i == 15:
                                # x fully queued; now the weight constants
                                nc.scalar.dma_start(
                                    w1t[:],
                                    w1[:].rearrange("(fb p) j -> p fb j", p=128),
                                )
                                nc.scalar.dma_start(
                                    w2t[:],
                                    w2[:].rearrange("(jb p) n -> p jb n", p=128),
                                )
                                nc.scalar.dma_start(b1t[:], b1T[:])
                                nc.scalar.dma_start(b2t[:], b2T[:])
                                nc.scalar.dma_start(vnt[:], vneg[:])
                                nc.scalar.dma_start(rrt[:], rrow[:])
                        at = adjp.tile(
                            [128, 4, SH], FP8, tag="adjA", bufs=6, name=f"aA{fh}{k4}"
                        )
                        nc.sync.dma_start(at[:], a_src[:, k4])
                        # fp8 DoubleRow: contraction 256 rows per matmul
                        # (ki = partition, ko = kk-pair), 2x FLOP rate.
                        for j2 in range(2):
                            q = k4 * 2 + j2
                            kb0 = k4 * 4 + 2 * j2
                            xt = xts[kb0 // 8]
                            o = kb0 % 8
                            for fx in range(2):
                                fb = 2 * fh + fx
                                lhs = xt[:, o : o + 2, fb * 128 : (fb + 1) * 128]
                                for mb in range(4):
                                    nc.tensor.matmul(
                                        ps[fx * 4 + mb][:],
                                        lhs,
                                        at[:, 2 * j2 : 2 * j2 + 2, mb * 512 : (mb + 1) * 512],
                                        start=(q == 0),
                                        stop=(q == 63),
                                        perf_mode=mybir.MatmulPerfMode.DoubleRow,
                                    )
                    for fx in range(2):
                        fb = 2 * fh + fx
                        for mb in range(4):
                            zz = ztp.tile(
                                [128, 512], BF16, tag="zt", bufs=16, name=f"zt{fb}{mb}"
                            )
                            nc.vector.tensor_copy(zz[:], ps[fx * 4 + mb][:])
                            zt[(fb, mb)] = zz

                for c in range(2):
                    # ---- phase H: hT chunk = relu(z @ W1 + b1)^T ----
                    for jbh in range(2):
                        psh = [
                            psum.tile([128, 512], F32, tag="ps", name=f"psH{c}{jbh}{i}")
                            for i in range(8)
                        ]
                        for jb in range(4):
                            jg = jbh * 4 + jb
                            for fb in range(4):
                                lhs = w1t[:, fb, jg * 128 : (jg + 1) * 128]
                                for mh in range(2):
                                    nc.tensor.matmul(
                                        psh[jb * 2 + mh][:],
                                        lhs,
                                        zt[(fb, c * 2 + mh)][:],
                                        start=(fb == 0),
                                        stop=False,
                                    )
                            for mh in range(2):
                                # rank-1 fp8-coherent-error correction (K=1)
                                mg = c * 2 + mh
                                nc.tensor.matmul(
                                    psh[jb * 2 + mh][:],
                                    vnt[0:1, jg * 128 : (jg + 1) * 128],
                                    rrt[0:1, mg * 512 : (mg + 1) * 512],
                                    start=False,
                                    stop=True,
                                )
                            for mh in range(2):
                                hh = htp.tile(
                                    [128, 512], BF16, tag="ht", bufs=16,
                                    name=f"ht{c}{jbh}{jb}{mh}",
                                )
                                nc.scalar.activation(
                                    hh[:],
                                    psh[jb * 2 + mh][:],
                                    mybir.ActivationFunctionType.Relu,
                                    bias=b1t[:, jg : jg + 1],
                                )
                                ht[(jg, c * 2 + mh)] = hh

                    # ---- phase S: s2 chunk = h @ W2; AG per quarter ----
                    for qq in (c * 2, c * 2 + 1):
                        for m4 in range(4):
                            pss = psum.tile([128, 256], F32, tag="ps", name=f"psS{qq}{m4}")
                            for jb in range(8):
                                nc.tensor.matmul(
                                    pss[:],
                                    ht[(jb, qq)][:, m4 * 128 : (m4 + 1) * 128],
                                    w2t[:, jb, :],
                                    start=(jb == 0),
                                    stop=(jb == 7),
                                )
                            so = smallp.tile([128, 256], FP8, tag="so", bufs=4)
                            nc.vector.tensor_scalar_mul(so[:], pss[:], S2_SCALE)
                            # SWDGE ring: keeps the ACT ring free so the
                            # adjD stream prefetches during phases A/H/S
                            nc.gpsimd.dma_start(
                                ag_in[qq][:, m4 * OUT : (m4 + 1) * OUT], so[:]
                            )
                        allgather(ag_in[qq], ag_out[qq])

            # ---- phase D: out2T = (adj_c @ s2)^T + b2 ----
            # All 8 psum banks accumulate concurrently; k-blocks consumed in
            # gather-arrival order (quarter-major), s2 tiles loaded JIT after
            # each adjT chunk so the SP queue stays load-ordered.
            with (
                tc.tile_pool(name="adjD", bufs=5) as adjDp,
                tc.tile_pool(name="s2p", bufs=6) as s2p,
                tc.tile_pool(name="outp", bufs=8) as outp,
            ):
                # ag_out[qq] row g*128+p, col skk*256+n  (rank g, block qq)
                s2srcs = [
                    ag_out[qq][:].rearrange("(g p) (skk n) -> p g skk n", p=128, n=OUT)
                    for qq in range(4)
                ]
                dsrc = adjD[:].rearrange("p (k4 kk m) -> p k4 kk m", k4=32, kk=4)
                dps = [
                    psum.tile([128, 512], F32, tag="ps", name=f"psD{i}")
                    for i in range(8)
                ]
                # k4 = g*4 + qq  ->  iterate quarter-major
                k4_order = [g * 4 + qq for qq in range(4) for g in range(8)]
                for ki, k4 in enumerate(k4_order):
                    g, qq = k4 // 4, k4 % 4
                    # adjD rides the ACT ring (idle after the x loads), so
                    # phase D's stream prefetches during phases A/H/S instead
                    # of queueing behind adjA on the SP ring.
                    at = adjDp.tile(
                        [128, 4, SH], FP8, tag="adjD", bufs=5, name=f"aD{k4}"
                    )
                    nc.scalar.dma_start(at[:], dsrc[:, k4])
                    st = s2p.tile([128, 4, OUT], FP8, tag="s2t", bufs=6, name=f"s2t{k4}")
                    nc.sync.dma_start(st[:], s2srcs[qq][:, g])
                    for j2 in range(2):
                        for n2t in range(2):
                            lhs = st[:, 2 * j2 : 2 * j2 + 2, n2t * 128 : (n2t + 1) * 128]
                            for mb in range(4):
                                nc.tensor.matmul(
                                    dps[n2t * 4 + mb][:],
                                    lhs,
                                    at[:, 2 * j2 : 2 * j2 + 2, mb * 512 : (mb + 1) * 512],
                                    start=(ki == 0 and j2 == 0),
                                    stop=(ki == 31 and j2 == 1),
                                    perf_mode=mybir.MatmulPerfMode.DoubleRow,
                                )
                # Final evac: scalar and vector engines each take half (they
                # can access PSUM concurrently on different banks), stores
                # split across both HWDGE rings — halves the serial tail
                # after the last matmul.
                inv = 1.0 / (ADJ_SCALE * S2_SCALE)
                for n2t in range(2):
                    for mb in range(4):
                        ot = outp.tile([128, 512], F32, tag="ot")
                        if mb % 2 == 0:
                            nc.scalar.activation(
                                ot[:],
                                dps[n2t * 4 + mb][:],
                                mybir.ActivationFunctionType.Identity,
                                bias=b2t[:, n2t : n2t + 1],
                                scale=inv,
                            )
                        else:
                            nc.vector.tensor_scalar(
                                ot[:],
                                dps[n2t * 4 + mb][:],
                                inv,
                                b2t[:, n2t : n2t + 1],
                                mybir.AluOpType.mult,
                                mybir.AluOpType.add,
                            )
                        dmaq = nc.scalar if mb % 2 == 0 else nc.sync
                        dmaq.dma_start(
                            out2T[
                                n2t * 128 : (n2t + 1) * 128, mb * 512 : (mb + 1) * 512
                            ],
                            ot[:],
                        )

    _elide_redundant_ldweights(nc)
    _split_excess_waits(nc)
    return nc


def _prep_inputs(x, adj, W1, b1, W2, b2):
    bf = ml_dtypes.bfloat16
    e4 = ml_dtypes.float8_e4m3
    # Phase A runs in fp8 (DoubleRow): adj pre-scaled by N into e4m3 range,
    # 1/N folded into W1.  x quantized to e4m3; the coherent part of its
    # quantization error (rank-1: adjq_rowsum x colmean(x_hi - x) @ W1s) is
    # cancelled on-device via a K=1 matmul with vneg/rrow.
    x_hi = x.astype(e4)
    # xP[p, kb*512+f] = x_hi[kb*128+p, f]
    xb = np.ascontiguousarray(
        x_hi.reshape(N // 128, 128, F).transpose(1, 0, 2)
    ).reshape(128, -1)
    w1s = (W1 / ADJ_SCALE).astype(bf)
    m = (x_hi.astype(np.float32) - x).mean(axis=0)  # [F]
    v = m @ w1s.astype(np.float32)  # [HID]
    vneg_ = np.ascontiguousarray((-v).reshape(1, HID)).astype(bf)
    w2b = W2.astype(bf)
    b1T = np.ascontiguousarray(b1.reshape(HID // 128, 128).T).astype(np.float32)
    b2T = np.ascontiguousarray(b2.reshape(OUT // 128, 128).T).astype(np.float32)
    in_maps = []
    for c in range(NCORES):
        rows = slice(c * SH, (c + 1) * SH)
        # adjT[k, m] = adj[c*SH + m, k], shape [N, SH], k-major
        adjTq = (np.ascontiguousarray(adj[rows, :].T) * ADJ_SCALE).astype(e4)
        d4 = adjTq.reshape(32, 4, 128, SH)  # [k4, kk, p, m]
        adjD_ = np.ascontiguousarray(d4.transpose(2, 0, 1, 3)).reshape(128, -1)
        rr = adjTq.astype(np.float32).sum(axis=0).reshape(1, SH)  # adjq rowsums
        in_maps.append(
            {
                "adjD": adjD_,
                "xP": xb,
                "w1": w1s,
                "w2": w2b,
                "b1T": b1T,
                "b2T": b2T,
                "vneg": vneg_,
                "rrow": np.ascontiguousarray(rr).astype(bf),
            }
        )
    return in_maps


def _run(inputs, trace=False):
    global _built
    if _built is None:
        _built = build()
    in_maps = _prep_inputs(**inputs)
    r = run_bass_kernel_spmd(_built, in_maps, list(range(NCORES)), trace=trace)
    out = np.empty([N, OUT], np.float32)
    for c in range(NCORES):
        out[c * SH : (c + 1) * SH, :] = r.results[c]["out2T"].T
    return out, r


def kernel(x, adj, W1, b1, W2, b2):
    out, _ = _run(dict(x=x, adj=adj, W1=W1, b1=b1, W2=W2, b2=b2))
    return out


# revision 43
# speedup vs baseline: 1.0635x; 1.0635x over previous
"""Trainium2 Bass kernel for a 2-layer dense GCN (NodeEncoder).

    out = adj @ relu(adj @ (x@W1) + b1) @ W2 + b2
    N=16384, F_IN=512, HID=1024, OUT=256, adj dense [N, N] fp32.

Key algebraic optimization vs the straightforward lowering: layer 1 is
computed as (adj @ x) @ W1 instead of adj @ (x @ W1).  The adj
contraction then runs against F_IN=512 columns instead of HID=1024,
halving the dominant matmul's FLOPs (275 vs 550 GFLOP), and since x is
replicated on every core the layer-1 AllGather disappears entirely.

Sharding: adj row-partitioned across 8 NeuronCores (2048 rows/core).
Per core (all matmuls bf16 with fp32 PSUM accumulation):

  phase A:  zT_c   = (adj_c @ x)^T          [512, 2048]   (lhsT = x
            blocks stationary, rhs = adjT_c streaming; out is zT)
  phase H:  hT_c   = relu(z_c @ W1 + b1)^T  [1024, 2048]  (lhsT = W1
            blocks, rhs = zT tiles; bias per-partition in ACT relu)
  phase S:  s2_c   = h_c @ W2               [2048, 256]   (lhsT = hT
            blocks, rhs = W2)
  AG:       s2     = AllGather(s2_c)        [16384, 256]  (in quarters,
            fired as soon as each quarter of s2_c is ready)
  phase D:  out2T_c = (adj_c @ s2)^T + b2   [256, 2048]   (lhsT = s2
            tiles, rhs = adjT_c streaming; b2 via ACT Identity)

Phases A/H/S are split in two m-chunks (1024 adj columns each) so the
first two AG quarters fire halfway through phase A and the gather
overlaps compute; phase D consumes k-blocks in gather-arrival order.
"""

import numpy as np
import ml_dtypes

import concourse.bass as bass
import concourse.mybir as mybir
import concourse.tile as tile
from concourse.bass_utils import run_bass_kernel_spmd
from concourse.tile_sem_assignment import N_PROCS
from concourse.vector_clock import ScopedClock, VectorClock

# ---------------------------------------------------------------------------
# Workaround: the walrus build in this container caps the number of sync-wait
# commands on a Drain instruction; Tile's kernel-tail drain aggregates one
# wait per logical processor and exceeds it.  Split the tail drain into a
# chain of single-wait drains on the same (SP) queue — semantically identical.
# ---------------------------------------------------------------------------


def _drain_and_barrier_split(self, tick_clock, wait_clock):
    gc = tick_clock.global_clock
    for p in range(N_PROCS):
        partial = VectorClock([gc[q] if q == p else 0 for q in range(N_PROCS)])
        d = self.nc.sync.drain()
        wait_clock.add_sem_waits(d.ins, ScopedClock({None: partial}))
    self.nc.sync.drain()

    self.nc.all_engine_barrier()
    assert self.sems is not None
    popped = self.nc._tile_sem_poison_stack.pop()
    assert popped is self._sem_poison
    self.nc.clear_and_free_semaphores(list(self.sems.allocated().values()))
    self.nc.all_engine_barrier()


tile.TileContext._drain_and_barrier = _drain_and_barrier_split

# The same walrus cap applies to every instruction kind: at most ONE sync
# wait command per instruction (probed empirically — a 2-wait TensorCopy is
# rejected).  Post-pass: hoist excess sem-waits onto no-ops inserted just
# before the instruction on the same engine queue — per-engine program order
# makes this semantically identical.
_MAX_WAITS = 1


def _split_excess_waits(nc):
    ctr = 0
    for f in nc.m.functions:
        for bb in f.blocks:
            out = []
            changed = False
            for inst in bb.instructions:
                si = inst.sync_info
                waits = list(si.on_wait) if si is not None and si.on_wait else []
                if len(waits) > _MAX_WAITS:
                    changed = True
                    keep, excess = waits[: _MAX_WAITS], waits[_MAX_WAITS :]
                    for i in range(0, len(excess), _MAX_WAITS):
                        ctr += 1
                        nop = mybir.InstNoOp(name=f"I-waitnop-{ctr}")
                        nop.engine = inst.engine
                        nop.sync_info = mybir.SyncInfo(
                            on_wait=excess[i : i + _MAX_WAITS], on_update=[]
                        )
                        out.append(nop)
                    si.on_wait = keep
                out.append(inst)
            if changed:
                bb.instructions = out
    return ctr


def _elide_redundant_ldweights(nc):
    """Delete an InstLdweights that reloads the exact weights AP loaded by
    the previous (surviving) InstLdweights when only plain matmuls / no-ops
    sit between them in the scheduled stream.  The PE array keeps the
    stationary operand across matmuls, so the reload is pure overhead
    (walrus emits one LDWEIGHTS per MATMUL and its ldw-opt pass is
    incompatible with pre-split LDW+MM).  Only sync-free LDWs are removed,
    so semaphore bookkeeping is unchanged."""
    n_elided = 0
    for f in nc.m.functions:
        for bb in f.blocks:
            out = []
            last_w = None  # weights-AP repr of last surviving LDW, if run intact
            changed = False
            for inst in bb.instructions:
                nm = type(inst).__name__
                if nm == "InstLdweights":
                    si = inst.sync_info
                    clean = not (si and (si.on_wait or si.on_update))
                    w = repr(inst.ins[0])
                    if clean and last_w == w:
                        n_elided += 1
                        changed = True
                        continue  # drop the reload
                    last_w = w if clean else None
                elif nm == "InstMatmult":
                    if getattr(inst, "is_transpose", False):
                        last_w = None
                elif nm == "InstNoOp":
                    pass
                else:
                    last_w = None
                out.append(inst)
            if changed:
                bb.instructions = out
    return n_elided


NCORES = 8
N = 16384
SH = N // NCORES  # 2048 adj rows per core
F = 512
HID = 1024
OUT = 256

BF16 = mybir.dt.bfloat16
F32 = mybir.dt.float32
FP8 = mybir.dt.float8e4
ADJ_SCALE = float(N)  # adj pre-scaled into fp8 range; 1/N folded into W1
S2_SCALE = 1024.0  # s2 pre-scaled into fp8 range; undone at phase D evac

_built = None


def build():
    """Build the per-core Bass program (identical on all cores)."""
    nc = bass.Bass()

    # All big inputs are host-prepped into partition-major tiled layouts so
    # every DMA reads long contiguous per-partition runs (8-64 KiB):
    #   adjD[p, (k4 kk m)] = adjT[k4*512+kk*128+p, m]  (phases A and D)
    #   xP  [p, (kb f)]    = x[kb*128+p, f]            (replicated)
    adjA = nc.declare_dram_parameter("adjA", [128, 2 * 32 * 4 * 1024], FP8, isOutput=False)
    adjD = nc.declare_dram_parameter("adjD", [128, 32 * 4 * SH], FP8, isOutput=False)
    xP = nc.declare_dram_parameter("xP", [128, (N // 128) * F], FP8, isOutput=False)
    w1 = nc.declare_dram_parameter("w1", [F, HID], BF16, isOutput=False)
    w2 = nc.declare_dram_parameter("w2", [HID, OUT], BF16, isOutput=False)
    b1T = nc.declare_dram_parameter("b1T", [128, HID // 128], F32, isOutput=False)
    b2T = nc.declare_dram_parameter("b2T", [128, OUT // 128], F32, isOutput=False)
    # rank-1 correction operands for the fp8 phase A (see _prep_inputs):
    #   pre1 += vneg^T . rrow   cancels the coherent x-quantization error
    vneg = nc.declare_dram_parameter("vneg", [1, HID], BF16, isOutput=False)
    rrow = nc.declare_dram_parameter("rrow", [1, SH], BF16, isOutput=False)
    out2T = nc.declare_dram_parameter("out2T", [OUT, SH], F32, isOutput=True)

    rg = [list(range(NCORES))]

    def allgather(inp, outp):
        return nc.gpsimd.collective_compute(
            "AllGather",
            mybir.AluOpType.bypass,
            replica_groups=rg,
            ins=[inp.opt()],
            outs=[outp.opt()],
        )

    with tile.TileContext(nc) as tc:
        with (
            tc.tile_pool(name="const", bufs=1) as constp,
            tc.tile_pool(name="psum", bufs=8, space="PSUM") as psum,
            tc.tile_pool(name="dram", bufs=1, space="DRAM") as dram,
        ):
            # ---- constants (ACT HWDGE ring; adj streams ride the SP ring).
            # Declared here, but the DMAs are issued AFTER the first x tiles
            # below: phase A's first matmul gates on x tile 0, while the
            # weights aren't read until phase H ~270us in.
            w1t = constp.tile([128, F // 128, HID], BF16)
            w2t = constp.tile([128, HID // 128, OUT], BF16)
            b1t = constp.tile([128, HID // 128], F32)
            b2t = constp.tile([128, OUT // 128], F32)
            vnt = constp.tile([1, HID], BF16)
            rrt = constp.tile([1, SH], BF16)

            # AG buffers partition-major: rank contribution [128, skk*256+n]
            # with s2 row skk*128+p; gathered output stacks ranks on dim 0.
            ag_in = [dram.tile([128, 4 * OUT], FP8, name=f"agi{q}") for q in range(4)]
            ag_out = [
                dram.tile([128 * 8, 4 * OUT], FP8, addr_space="Shared", name=f"ago{q}")
                for q in range(4)
            ]

            xsrc = xP[:].rearrange("p (kb f) -> p kb f", f=F)

            with (
                tc.tile_pool(name="xp", bufs=1) as xp,
                tc.tile_pool(name="zt", bufs=16) as ztp,
                tc.tile_pool(name="ht", bufs=16) as htp,
                tc.tile_pool(name="adjA", bufs=6) as adjp,
                tc.tile_pool(name="small", bufs=4) as smallp,
            ):
                xts = []  # 16 tiles of 8 k-blocks each
                zt = {}
                ht = {}
                aAv = adjA[:].rearrange(
                    "p (c k4 kk m) -> p c k4 kk m", c=2, k4=32, kk=4
                )
                for c in range(2):
                    asrc = aAv[:, c]
                    # ---- phase A: zT chunk = (adj_c @ x)^T cols c*1024.. ----
                    ps = [
                        psum.tile([128, 512], F32, tag="ps", name=f"psA{c}{i}")
                        for i in range(8)
                    ]
                    for k4 in range(32):
                        if c == 0 and k4 % 2 == 0:
                            i = k4 // 2
                            t = xp.tile([128, 8, F], FP8, name=f"xt{i}")
                            nc.scalar.dma_start(t[:], xsrc[:, i * 8 : (i + 1) * 8])
                            xts.append(t)
                            if i == 15:
                                # x fully queued; now the weight constants
                                nc.scalar.dma_start(
                                    w1t[:],
                                    w1[:].rearrange("(fb p) j -> p fb j", p=128),
                                )
                                nc.scalar.dma_start(
                                    w2t[:],
                                    w2[:].rearrange("(jb p) n -> p jb n", p=128),
                                )
                                nc.scalar.dma_start(b1t[:], b1T[:])
                                nc.scalar.dma_start(b2t[:], b2T[:])
                                nc.scalar.dma_start(vnt[:], vneg[:])
                                nc.scalar.dma_start(rrt[:], rrow[:])
                        at = adjp.tile(
                            [128, 4, 1024], FP8, tag="adjA", bufs=6, name=f"aA{c}{k4}"
                        )
                        nc.sync.dma_start(at[:], asrc[:, k4])
                        # fp8 DoubleRow: contraction 256 rows per matmul
                        # (ki = partition, ko = kk-pair), 2x FLOP rate.
                        for j2 in range(2):
                            q = k4 * 2 + j2
                            kb0 = k4 * 4 + 2 * j2
                            xt = xts[kb0 // 8]
                            o = kb0 % 8
                            for fb in range(4):
                                lhs = xt[:, o : o + 2, fb * 128 : (fb + 1) * 128]
                                for mh in range(2):
                                    nc.tensor.matmul(
                                        ps[fb * 2 + mh][:],
                                        lhs,
                                        at[:, 2 * j2 : 2 * j2 + 2, mh * 512 : (mh + 1) * 512],
                                        start=(q == 0),
                                        stop=(q == 63),
                                        perf_mode=mybir.MatmulPerfMode.DoubleRow,
                                    )
                    for fb in range(4):
                        for mh in range(2):
                            zz = ztp.tile(
                                [128, 512], BF16, tag="zt", bufs=16,
                                name=f"zt{c}{fb}{mh}",
                            )
                            nc.vector.tensor_copy(zz[:], ps[fb * 2 + mh][:])
                            zt[(fb, c * 2 + mh)] = zz

                    # ---- phase H: hT chunk = relu(z @ W1 + b1)^T ----
                    for jbh in range(2):
                        psh = [
                            psum.tile([128, 512], F32, tag="ps", name=f"psH{c}{jbh}{i}")
                            for i in range(8)
                        ]
                        for jb in range(4):
                            jg = jbh * 4 + jb
                            for fb in range(4):
                                lhs = w1t[:, fb, jg * 128 : (jg + 1) * 128]
                                for mh in range(2):
                                    nc.tensor.matmul(
                                        psh[jb * 2 + mh][:],
                                        lhs,
                                        zt[(fb, c * 2 + mh)][:],
                                        start=(fb == 0),
                                        stop=False,
                                    )
                            for mh in range(2):
                                # rank-1 fp8-coherent-error correction (K=1)
                                mg = c * 2 + mh
                                nc.tensor.matmul(
                                    psh[jb * 2 + mh][:],
                                    vnt[0:1, jg * 128 : (jg + 1) * 128],
                                    rrt[0:1, mg * 512 : (mg + 1) * 512],
                                    start=False,
                                    stop=True,
                                )
                            for mh in range(2):
                                hh = htp.tile(
                                    [128, 512], BF16, tag="ht", bufs=16,
                                    name=f"ht{c}{jbh}{jb}{mh}",
                                )
                                nc.scalar.activation(
                                    hh[:],
                                    psh[jb * 2 + mh][:],
                                    mybir.ActivationFunctionType.Relu,
                                    bias=b1t[:, jg : jg + 1],
                                )
                                ht[(jg, c * 2 + mh)] = hh

                    # ---- phase S: s2 chunk = h @ W2; AG per quarter ----
                    for qq in (c * 2, c * 2 + 1):
                        for m4 in range(4):
                            pss = psum.tile([128, 256], F32, tag="ps", name=f"psS{qq}{m4}")
                            for jb in range(8):
                                nc.tensor.matmul(
                                    pss[:],
                                    ht[(jb, qq)][:, m4 * 128 : (m4 + 1) * 128],
                                    w2t[:, jb, :],
                                    start=(jb == 0),
                                    stop=(jb == 7),
                                )
                            so = smallp.tile([128, 256], FP8, tag="so", bufs=4)
                            nc.vector.tensor_scalar_mul(so[:], pss[:], S2_SCALE)
                            # SWDGE ring: keeps the ACT ring free so the
                            # adjD stream prefetches during phases A/H/S
                            nc.gpsimd.dma_start(
                                ag_in[qq][:, m4 * OUT : (m4 + 1) * OUT], so[:]
                            )
                        allgather(ag_in[qq], ag_out[qq])

            # ---- phase D: out2T = (adj_c @ s2)^T + b2 ----
            # All 8 psum banks accumulate concurrently; k-blocks consumed in
            # gather-arrival order (quarter-major), s2 tiles loaded JIT after
            # each adjT chunk so the SP queue stays load-ordered.
            with (
                tc.tile_pool(name="adjD", bufs=5) as adjDp,
                tc.tile_pool(name="s2p", bufs=6) as s2p,
                tc.tile_pool(name="outp", bufs=8) as outp,
            ):
                # ag_out[qq] row g*128+p, col skk*256+n  (rank g, block qq)
                s2srcs = [
                    ag_out[qq][:].rearrange("(g p) (skk n) -> p g skk n", p=128, n=OUT)
                    for qq in range(4)
                ]
                dsrc = adjD[:].rearrange("p (k4 kk m) -> p k4 kk m", k4=32, kk=4)
                dps = [
                    psum.tile([128, 512], F32, tag="ps", name=f"psD{i}")
                    for i in range(8)
                ]
                # k4 = g*4 + qq  ->  iterate quarter-major
                k4_order = [g * 4 + qq for qq in range(4) for g in range(8)]
                for ki, k4 in enumerate(k4_order):
                    g, qq = k4 // 4, k4 % 4
                    # adjD rides the ACT ring (idle after the x loads), so
                    # phase D's stream prefetches during phases A/H/S instead
                    # of queueing behind adjA on the SP ring.
                    at = adjDp.tile(
                        [128, 4, SH], FP8, tag="adjD", bufs=5, name=f"aD{k4}"
                    )
                    nc.scalar.dma_start(at[:], dsrc[:, k4])
                    st = s2p.tile([128, 4, OUT], FP8, tag="s2t", bufs=6, name=f"s2t{k4}")
                    nc.sync.dma_start(st[:], s2srcs[qq][:, g])
                    for j2 in range(2):
                        for n2t in range(2):
                            lhs = st[:, 2 * j2 : 2 * j2 + 2, n2t * 128 : (n2t + 1) * 128]
                            for mb in range(4):
                                nc.tensor.matmul(
                                    dps[n2t * 4 + mb][:],
                                    lhs,
                                    at[:, 2 * j2 : 2 * j2 + 2, mb * 512 : (mb + 1) * 512],
                                    start=(ki == 0 and j2 == 0),
                                    stop=(ki == 31 and j2 == 1),
                                    perf_mode=mybir.MatmulPerfMode.DoubleRow,
                                )
                # Final evac: scalar and vector engines each take half (they
                # can access PSUM concurrently on different banks), stores
                # split across both HWDGE rings — halves the serial tail
                # after the last matmul.
                inv = 1.0 / (ADJ_SCALE * S2_SCALE)
                for n2t in range(2):
                    for mb in range(4):
                        ot = outp.tile([128, 512], F32, tag="ot")
                        if mb % 2 == 0:
                            nc.scalar.activation(
                                ot[:],
                                dps[n2t * 4 + mb][:],
                                mybir.ActivationFunctionType.Identity,
                                bias=b2t[:, n2t : n2t + 1],
                                scale=inv,
                            )
                        else:
                            nc.vector.tensor_scalar(
                                ot[:],
                                dps[n2t * 4 + mb][:],
                                inv,
                                b2t[:, n2t : n2t + 1],
                                mybir.AluOpType.mult,
                                mybir.AluOpType.add,
                            )
                        dmaq = nc.scalar if mb % 2 == 0 else nc.sync
                        dmaq.dma_start(
                            out2T[
                                n2t * 128 : (n2t + 1) * 128, mb * 512 : (mb + 1) * 512
                            ],
                            ot[:],
                        )

    _elide_redundant_ldweights(nc)
    _split_excess_waits(nc)
    return nc


def _prep_inputs(x, adj, W1, b1, W2, b2):
    bf = ml_dtypes.bfloat16
    e4 = ml_dtypes.float8_e4m3
    # Phase A runs in fp8 (DoubleRow): adj pre-scaled by N into e4m3 range,
    # 1/N folded into W1.  x quantized to e4m3; the coherent part of its
    # quantization error (rank-1: adjq_rowsum x colmean(x_hi - x) @ W1s) is
    # cancelled on-device via a K=1 matmul with vneg/rrow.
    x_hi = x.astype(e4)
    # xP[p, kb*512+f] = x_hi[kb*128+p, f]
    xb = np.ascontiguousarray(
        x_hi.reshape(N // 128, 128, F).transpose(1, 0, 2)
    ).reshape(128, -1)
    w1s = (W1 / ADJ_SCALE).astype(bf)
    m = (x_hi.astype(np.float32) - x).mean(axis=0)  # [F]
    v = m @ w1s.astype(np.float32)  # [HID]
    vneg_ = np.ascontiguousarray((-v).reshape(1, HID)).astype(bf)
    w2b = W2.astype(bf)
    b1T = np.ascontiguousarray(b1.reshape(HID // 128, 128).T).astype(np.float32)
    b2T = np.ascontiguousarray(b2.reshape(OUT // 128, 128).T).astype(np.float32)
    in_maps = []
    for c in range(NCORES):
        rows = slice(c * SH, (c + 1) * SH)
        # adjT[k, m] = adj[c*SH + m, k], shape [N, SH], k-major
        adjTq = (np.ascontiguousarray(adj[rows, :].T) * ADJ_SCALE).astype(e4)
        a5 = adjTq.reshape(32, 4, 128, 2, 1024)  # [k4, kk, p, cchunk, m]
        adjA_ = np.ascontiguousarray(a5.transpose(2, 3, 0, 1, 4)).reshape(128, -1)
        d4 = adjTq.reshape(32, 4, 128, SH)  # [k4, kk, p, m]
        adjD_ = np.ascontiguousarray(d4.transpose(2, 0, 1, 3)).reshape(128, -1)
        rr = adjTq.astype(np.float32).sum(axis=0).reshape(1, SH)  # adjq rowsums
        in_maps.append(
            {
                "adjA": adjA_,
                "adjD": adjD_,
                "xP": xb,
                "w1": w1s,
                "w2": w2b,
                "b1T": b1T,
                "b2T": b2T,
                "vneg": vneg_,
                "rrow": np.ascontiguousarray(rr).astype(bf),
            }
        )
    return in_maps


def _run(inputs, trace=False):
    global _built
    if _built is None:
        _built = build()
    in_maps = _prep_inputs(**inputs)
    r = run_bass_kernel_spmd(_built, in_maps, list(range(NCORES)), trace=trace)
    out = np.empty([N, OUT], np.float32)
    for c in range(NCORES):
        out[c * SH : (c + 1) * SH, :] = r.results[c]["out2T"].T
    return out, r


def kernel(x, adj, W1, b1, W2, b2):
    out, _ = _run(dict(x=x, adj=adj, W1=W1, b1=b1, W2=W2, b2=b2))
    return out


# revision 47
# speedup vs baseline: 1.1005x; 1.0348x over previous
"""Trainium2 Bass kernel for a 2-layer dense GCN (NodeEncoder).

    out = adj @ relu(adj @ (x@W1) + b1) @ W2 + b2
    N=16384, F_IN=512, HID=1024, OUT=256, adj dense [N, N] fp32.

Key algebraic optimization vs the straightforward lowering: layer 1 is
computed as (adj @ x) @ W1 instead of adj @ (x @ W1).  The adj
contraction then runs against F_IN=512 columns instead of HID=1024,
halving the dominant matmul's FLOPs (275 vs 550 GFLOP), and since x is
replicated on every core the layer-1 AllGather disappears entirely.

Sharding: adj row-partitioned across 8 NeuronCores (2048 rows/core).
Per core (all matmuls bf16 with fp32 PSUM accumulation):

  phase A:  zT_c   = (adj_c @ x)^T          [512, 2048]   (lhsT = x
            blocks stationary, rhs = adjT_c streaming; out is zT)
  phase H:  hT_c   = relu(z_c @ W1 + b1)^T  [1024, 2048]  (lhsT = W1
            blocks, rhs = zT tiles; bias per-partition in ACT relu)
  phase S:  s2_c   = h_c @ W2               [2048, 256]   (lhsT = hT
            blocks, rhs = W2)
  AG:       s2     = AllGather(s2_c)        [16384, 256]  (in quarters,
            fired as soon as each quarter of s2_c is ready)
  phase D:  out2T_c = (adj_c @ s2)^T + b2   [256, 2048]   (lhsT = s2
            tiles, rhs = adjT_c streaming; b2 via ACT Identity)

Phases A/H/S are split in two m-chunks (1024 adj columns each) so the
first two AG quarters fire halfway through phase A and the gather
overlaps compute; phase D consumes k-blocks in gather-arrival order.
"""

import numpy as np
import ml_dtypes

import concourse.bass as bass
import concourse.mybir as mybir
import concourse.tile as tile
from concourse.bass_utils import run_bass_kernel_spmd
from concourse.tile_sem_assignment import N_PROCS
from concourse.vector_clock import ScopedClock, VectorClock

# ---------------------------------------------------------------------------
# Workaround: the walrus build in this container caps the number of sync-wait
# commands on a Drain instruction; Tile's kernel-tail drain aggregates one
# wait per logical processor and exceeds it.  Split the tail drain into a
# chain of single-wait drains on the same (SP) queue — semantically identical.
# ---------------------------------------------------------------------------


def _drain_and_barrier_split(self, tick_clock, wait_clock):
    gc = tick_clock.global_clock
    for p in range(N_PROCS):
        partial = VectorClock([gc[q] if q == p else 0 for q in range(N_PROCS)])
        d = self.nc.sync.drain()
        wait_clock.add_sem_waits(d.ins, ScopedClock({None: partial}))
    self.nc.sync.drain()

    self.nc.all_engine_barrier()
    assert self.sems is not None
    popped = self.nc._tile_sem_poison_stack.pop()
    assert popped is self._sem_poison
    self.nc.clear_and_free_semaphores(list(self.sems.allocated().values()))
    self.nc.all_engine_barrier()


tile.TileContext._drain_and_barrier = _drain_and_barrier_split

# The same walrus cap applies to every instruction kind: at most ONE sync
# wait command per instruction (probed empirically — a 2-wait TensorCopy is
# rejected).  Post-pass: hoist excess sem-waits onto no-ops inserted just
# before the instruction on the same engine queue — per-engine program order
# makes this semantically identical.
_MAX_WAITS = 1


def _split_excess_waits(nc):
    ctr = 0
    for f in nc.m.functions:
        for bb in f.blocks:
            out = []
            changed = False
            for inst in bb.instructions:
                si = inst.sync_info
                waits = list(si.on_wait) if si is not None and si.on_wait else []
                if len(waits) > _MAX_WAITS:
                    changed = True
                    keep, excess = waits[: _MAX_WAITS], waits[_MAX_WAITS :]
                    for i in range(0, len(excess), _MAX_WAITS):
                        ctr += 1
                        nop = mybir.InstNoOp(name=f"I-waitnop-{ctr}")
                        nop.engine = inst.engine
                        nop.sync_info = mybir.SyncInfo(
                            on_wait=excess[i : i + _MAX_WAITS], on_update=[]
                        )
                        out.append(nop)
                    si.on_wait = keep
                out.append(inst)
            if changed:
                bb.instructions = out
    return ctr


def _elide_redundant_ldweights(nc):
    """Delete an InstLdweights that reloads the exact weights AP loaded by
    the previous (surviving) InstLdweights when only plain matmuls / no-ops
    sit between them in the scheduled stream.  The PE array keeps the
    stationary operand across matmuls, so the reload is pure overhead
    (walrus emits one LDWEIGHTS per MATMUL and its ldw-opt pass is
    incompatible with pre-split LDW+MM).  Only sync-free LDWs are removed,
    so semaphore bookkeeping is unchanged."""
    n_elided = 0
    for f in nc.m.functions:
        for bb in f.blocks:
            out = []
            last_w = None  # weights-AP repr of last surviving LDW, if run intact
            changed = False
            for inst in bb.instructions:
                nm = type(inst).__name__
                if nm == "InstLdweights":
                    si = inst.sync_info
                    clean = not (si and (si.on_wait or si.on_update))
                    w = repr(inst.ins[0])
                    if clean and last_w == w:
                        n_elided += 1
                        changed = True
                        continue  # drop the reload
                    last_w = w if clean else None
                elif nm == "InstMatmult":
                    if getattr(inst, "is_transpose", False):
                        last_w = None
                elif nm == "InstNoOp":
                    pass
                else:
                    last_w = None
                out.append(inst)
            if changed:
                bb.instructions = out
    return n_elided


NCORES = 8
N = 16384
SH = N // NCORES  # 2048 adj rows per core
F = 512
HID = 1024
OUT = 256

BF16 = mybir.dt.bfloat16
F32 = mybir.dt.float32
FP8 = mybir.dt.float8e4
ADJ_SCALE = float(N)  # adj pre-scaled into fp8 range; 1/N folded into W1
S2_SCALE = 1024.0  # s2 pre-scaled into fp8 range; undone at phase D evac

_built = None


def build():
    """Build the per-core Bass program (identical on all cores)."""
    nc = bass.Bass()

    # All big inputs are host-prepped into partition-major tiled layouts so
    # every DMA reads long contiguous per-partition runs (8-64 KiB):
    #   adjD[p, (k4 kk m)] = adjT[k4*512+kk*128+p, m]  (phases A and D)
    #   xP  [p, (kb f)]    = x[kb*128+p, f]            (replicated)
    adjA = nc.declare_dram_parameter("adjA", [128, 2 * 32 * 4 * 1024], FP8, isOutput=False)
    adjD = nc.declare_dram_parameter("adjD", [128, 32 * 4 * SH], FP8, isOutput=False)
    xP = nc.declare_dram_parameter("xP", [128, (N // 128) * F], FP8, isOutput=False)
    w1 = nc.declare_dram_parameter("w1", [F, HID], BF16, isOutput=False)
    w2 = nc.declare_dram_parameter("w2", [HID, OUT], BF16, isOutput=False)
    b1T = nc.declare_dram_parameter("b1T", [128, HID // 128], F32, isOutput=False)
    b2T = nc.declare_dram_parameter("b2T", [128, OUT // 128], F32, isOutput=False)
    # rank-1 correction operands for the fp8 phase A (see _prep_inputs):
    #   pre1 += vneg^T . rrow   cancels the coherent x-quantization error
    vneg = nc.declare_dram_parameter("vneg", [1, HID], BF16, isOutput=False)
    rrow = nc.declare_dram_parameter("rrow", [1, SH], BF16, isOutput=False)
    out2T = nc.declare_dram_parameter("out2T", [OUT, SH], F32, isOutput=True)

    rg = [list(range(NCORES))]

    def allgather(inp, outp):
        return nc.gpsimd.collective_compute(
            "AllGather",
            mybir.AluOpType.bypass,
            replica_groups=rg,
            ins=[inp.opt()],
            outs=[outp.opt()],
        )

    with tile.TileContext(nc) as tc:
        with (
            tc.tile_pool(name="const", bufs=1) as constp,
            tc.tile_pool(name="psum", bufs=8, space="PSUM") as psum,
            tc.tile_pool(name="dram", bufs=1, space="DRAM") as dram,
        ):
            # ---- constants (ACT HWDGE ring; adj streams ride the SP ring).
            # Declared here, but the DMAs are issued AFTER the first x tiles
            # below: phase A's first matmul gates on x tile 0, while the
            # weights aren't read until phase H ~270us in.
            w1t = constp.tile([128, F // 128, HID], BF16)
            w2t = constp.tile([128, HID // 128, OUT], BF16)
            b1t = constp.tile([128, HID // 128], F32)
            b2t = constp.tile([128, OUT // 128], F32)
            vnt = constp.tile([1, HID], BF16)
            rrt = constp.tile([1, SH], BF16)

            # AG buffers partition-major: rank contribution [128, skk*256+n]
            # with s2 row skk*128+p; gathered output stacks ranks on dim 0.
            ag_in = [dram.tile([128, 4 * OUT], FP8, name=f"agi{q}") for q in range(4)]
            ag_out = [
                dram.tile([128 * 8, 4 * OUT], FP8, addr_space="Shared", name=f"ago{q}")
                for q in range(4)
            ]

            # xP holds SW-interleaved stationary blocks for phase A:
            #   [p, q, fb, (127-c)*2 + ko] = x_hi[q*256 + ko*128 + p, fb*128 + c]
            xsrc = xP[:].rearrange("p (q fb c) -> p q fb c", q=N // 256, fb=F // 128)

            with (
                tc.tile_pool(name="xp", bufs=1) as xp,
                tc.tile_pool(name="zt", bufs=16) as ztp,
                tc.tile_pool(name="ht", bufs=16) as htp,
                tc.tile_pool(name="adjA", bufs=6) as adjp,
                tc.tile_pool(name="small", bufs=4) as smallp,
            ):
                xts = []  # 16 tiles of 8 k-blocks each
                zt = {}
                ht = {}
                aAv = adjA[:].rearrange(
                    "p (c k4 kk m) -> p c k4 kk m", c=2, k4=32, kk=4
                )
                for c in range(2):
                    asrc = aAv[:, c]
                    # ---- phase A: zT chunk = (adj_c @ x)^T cols c*1024.. ----
                    ps = [
                        psum.tile([128, 512], F32, tag="ps", name=f"psA{c}{i}")
                        for i in range(8)
                    ]
                    for k4 in range(32):
                        if c == 0 and k4 % 2 == 0:
                            i = k4 // 2
                            t = xp.tile([128, 4, 4, 256], FP8, name=f"xt{i}")
                            nc.scalar.dma_start(t[:], xsrc[:, i * 4 : (i + 1) * 4])
                            xts.append(t)
                            if i == 15:
                                # x fully queued; now the weight constants
                                nc.scalar.dma_start(
                                    w1t[:],
                                    w1[:].rearrange("(fb p) j -> p fb j", p=128),
                                )
                                nc.scalar.dma_start(
                                    w2t[:],
                                    w2[:].rearrange("(jb p) n -> p jb n", p=128),
                                )
                                nc.scalar.dma_start(b1t[:], b1T[:])
                                nc.scalar.dma_start(b2t[:], b2T[:])
                                nc.scalar.dma_start(vnt[:], vneg[:])
                                nc.scalar.dma_start(rrt[:], rrow[:])
                        at = adjp.tile(
                            [128, 4, 1024], FP8, tag="adjA", bufs=6, name=f"aA{c}{k4}"
                        )
                        nc.sync.dma_start(at[:], asrc[:, k4])
                        # fp8 DoubleRow: contraction 256 rows per matmul
                        # (ki = partition, ko = kk-pair), 2x FLOP rate.
                        for j2 in range(2):
                            q = k4 * 2 + j2
                            xt = xts[q // 4]
                            for fb in range(4):
                                lhs = xt[:, q % 4, fb, :]
                                for mh in range(2):
                                    nc.tensor.matmul(
                                        ps[fb * 2 + mh][:],
                                        lhs,
                                        at[:, 2 * j2 : 2 * j2 + 2, mh * 512 : (mh + 1) * 512],
                                        start=(q == 0),
                                        stop=(q == 63),
                                        perf_mode=mybir.MatmulPerfMode.DoubleRowSwInterleave,
                                    )
                    for fb in range(4):
                        for mh in range(2):
                            zz = ztp.tile(
                                [128, 512], BF16, tag="zt", bufs=16,
                                name=f"zt{c}{fb}{mh}",
                            )
                            nc.vector.tensor_copy(zz[:], ps[fb * 2 + mh][:])
                            zt[(fb, c * 2 + mh)] = zz

                    # ---- phase H: hT chunk = relu(z @ W1 + b1)^T ----
                    for jbh in range(2):
                        psh = [
                            psum.tile([128, 512], F32, tag="ps", name=f"psH{c}{jbh}{i}")
                            for i in range(8)
                        ]
                        for jb in range(4):
                            jg = jbh * 4 + jb
                            for fb in range(4):
                                lhs = w1t[:, fb, jg * 128 : (jg + 1) * 128]
                                for mh in range(2):
                                    nc.tensor.matmul(
                                        psh[jb * 2 + mh][:],
                                        lhs,
                                        zt[(fb, c * 2 + mh)][:],
                                        start=(fb == 0),
                                        stop=False,
                                    )
                            for mh in range(2):
                                # rank-1 fp8-coherent-error correction (K=1)
                                mg = c * 2 + mh
                                nc.tensor.matmul(
                                    psh[jb * 2 + mh][:],
                                    vnt[0:1, jg * 128 : (jg + 1) * 128],
                                    rrt[0:1, mg * 512 : (mg + 1) * 512],
                                    start=False,
                                    stop=True,
                                )
                            for mh in range(2):
                                hh = htp.tile(
                                    [128, 512], BF16, tag="ht", bufs=16,
                                    name=f"ht{c}{jbh}{jb}{mh}",
                                )
                                nc.scalar.activation(
                                    hh[:],
                                    psh[jb * 2 + mh][:],
                                    mybir.ActivationFunctionType.Relu,
                                    bias=b1t[:, jg : jg + 1],
                                )
                                ht[(jg, c * 2 + mh)] = hh

                    # ---- phase S: s2 chunk = h @ W2; AG per quarter ----
                    for qq in (c * 2, c * 2 + 1):
                        for m4 in range(4):
                            pss = psum.tile([128, 256], F32, tag="ps", name=f"psS{qq}{m4}")
                            for jb in range(8):
                                nc.tensor.matmul(
                                    pss[:],
                                    ht[(jb, qq)][:, m4 * 128 : (m4 + 1) * 128],
                                    w2t[:, jb, :],
                                    start=(jb == 0),
                                    stop=(jb == 7),
                                )
                            so = smallp.tile([128, 256], FP8, tag="so", bufs=4)
                            nc.vector.tensor_scalar_mul(so[:], pss[:], S2_SCALE)
                            # SWDGE ring: keeps the ACT ring free so the
                            # adjD stream prefetches during phases A/H/S
                            nc.gpsimd.dma_start(
                                ag_in[qq][:, m4 * OUT : (m4 + 1) * OUT], so[:]
                            )
                        allgather(ag_in[qq], ag_out[qq])

            # ---- phase D: out2T = (adj_c @ s2)^T + b2 ----
            # All 8 psum banks accumulate concurrently; k-blocks consumed in
            # gather-arrival order (quarter-major), s2 tiles loaded JIT after
            # each adjT chunk so the SP queue stays load-ordered.
            with (
                tc.tile_pool(name="adjD", bufs=5) as adjDp,
                tc.tile_pool(name="s2p", bufs=6) as s2p,
                tc.tile_pool(name="outp", bufs=8) as outp,
            ):
                # ag_out[qq] row g*128+p, col skk*256+n  (rank g, block qq)
                s2srcs = [
                    ag_out[qq][:].rearrange("(g p) (skk n) -> p g skk n", p=128, n=OUT)
                    for qq in range(4)
                ]
                dsrc = adjD[:].rearrange("p (k4 kk m) -> p k4 kk m", k4=32, kk=4)
                dps = [
                    psum.tile([128, 512], F32, tag="ps", name=f"psD{i}")
                    for i in range(8)
                ]
                # k4 = g*4 + qq  ->  iterate quarter-major
                k4_order = [g * 4 + qq for qq in range(4) for g in range(8)]
                for ki, k4 in enumerate(k4_order):
                    g, qq = k4 // 4, k4 % 4
                    # adjD rides the ACT ring (idle after the x loads), so
                    # phase D's stream prefetches during phases A/H/S instead
                    # of queueing behind adjA on the SP ring.
                    at = adjDp.tile(
                        [128, 4, SH], FP8, tag="adjD", bufs=5, name=f"aD{k4}"
                    )
                    nc.scalar.dma_start(at[:], dsrc[:, k4])
                    st = s2p.tile([128, 4, OUT], FP8, tag="s2t", bufs=6, name=f"s2t{k4}")
                    nc.sync.dma_start(st[:], s2srcs[qq][:, g])
                    for j2 in range(2):
                        for n2t in range(2):
                            lhs = st[:, 2 * j2 : 2 * j2 + 2, n2t * 128 : (n2t + 1) * 128]
                            for mb in range(4):
                                nc.tensor.matmul(
                                    dps[n2t * 4 + mb][:],
                                    lhs,
                                    at[:, 2 * j2 : 2 * j2 + 2, mb * 512 : (mb + 1) * 512],
                                    start=(ki == 0 and j2 == 0),
                                    stop=(ki == 31 and j2 == 1),
                                    perf_mode=mybir.MatmulPerfMode.DoubleRow,
                                )
                # Final evac: scalar and vector engines each take half (they
                # can access PSUM concurrently on different banks), stores
                # split across both HWDGE rings — halves the serial tail
                # after the last matmul.
                inv = 1.0 / (ADJ_SCALE * S2_SCALE)
                for n2t in range(2):
                    for mb in range(4):
                        ot = outp.tile([128, 512], F32, tag="ot")
                        if mb % 2 == 0:
                            nc.scalar.activation(
                                ot[:],
                                dps[n2t * 4 + mb][:],
                                mybir.ActivationFunctionType.Identity,
                                bias=b2t[:, n2t : n2t + 1],
                                scale=inv,
                            )
                        else:
                            nc.vector.tensor_scalar(
                                ot[:],
                                dps[n2t * 4 + mb][:],
                                inv,
                                b2t[:, n2t : n2t + 1],
                                mybir.AluOpType.mult,
                                mybir.AluOpType.add,
                            )
                        dmaq = nc.scalar if mb % 2 == 0 else nc.sync
                        dmaq.dma_start(
                            out2T[
                                n2t * 128 : (n2t + 1) * 128, mb * 512 : (mb + 1) * 512
                            ],
                            ot[:],
                        )

    _elide_redundant_ldweights(nc)
    _split_excess_waits(nc)
    return nc


def _prep_inputs(x, adj, W1, b1, W2, b2):
    bf = ml_dtypes.bfloat16
    e4 = ml_dtypes.float8_e4m3
    # Phase A runs in fp8 (DoubleRow): adj pre-scaled by N into e4m3 range,
    # 1/N folded into W1.  x quantized to e4m3; the coherent part of its
    # quantization error (rank-1: adjq_rowsum x colmean(x_hi - x) @ W1s) is
    # cancelled on-device via a K=1 matmul with vneg/rrow.
    x_hi = x.astype(e4)
    # SW-interleaved stationary layout for DoubleRowSwInterleave:
    #   xP[p, q, fb, (127-c)*2+ko] = x_hi[q*256 + ko*128 + p, fb*128 + c]
    a5x = x_hi.reshape(N // 256, 2, 128, F // 128, 128)  # [q, ko, ki, fb, c]
    xb = np.ascontiguousarray(
        a5x.transpose(2, 0, 3, 4, 1)[:, :, :, ::-1, :]
    ).reshape(128, -1)
    w1s = (W1 / ADJ_SCALE).astype(bf)
    m = (x_hi.astype(np.float32) - x).mean(axis=0)  # [F]
    v = m @ w1s.astype(np.float32)  # [HID]
    vneg_ = np.ascontiguousarray((-v).reshape(1, HID)).astype(bf)
    w2b = W2.astype(bf)
    b1T = np.ascontiguousarray(b1.reshape(HID // 128, 128).T).astype(np.float32)
    b2T = np.ascontiguousarray(b2.reshape(OUT // 128, 128).T).astype(np.float32)
    in_maps = []
    for c in range(NCORES):
        rows = slice(c * SH, (c + 1) * SH)
        # adjT[k, m] = adj[c*SH + m, k], shape [N, SH], k-major
        adjTq = (np.ascontiguousarray(adj[rows, :].T) * ADJ_SCALE).astype(e4)
        a5 = adjTq.reshape(32, 4, 128, 2, 1024)  # [k4, kk, p, cchunk, m]
        adjA_ = np.ascontiguousarray(a5.transpose(2, 3, 0, 1, 4)).reshape(128, -1)
        d4 = adjTq.reshape(32, 4, 128, SH)  # [k4, kk, p, m]
        adjD_ = np.ascontiguousarray(d4.transpose(2, 0, 1, 3)).reshape(128, -1)
        rr = adjTq.astype(np.float32).sum(axis=0).reshape(1, SH)  # adjq rowsums
        in_maps.append(
            {
                "adjA": adjA_,
                "adjD": adjD_,
                "xP": xb,
                "w1": w1s,
                "w2": w2b,
                "b1T": b1T,
                "b2T": b2T,
                "vneg": vneg_,
                "rrow": np.ascontiguousarray(rr).astype(bf),
            }
        )
    return in_maps


def _run(inputs, trace=False):
    global _built
    if _built is None:
        _built = build()
    in_maps = _prep_inputs(**inputs)
    r = run_bass_kernel_spmd(_built, in_maps, list(range(NCORES)), trace=trace)
    out = np.empty([N, OUT], np.float32)
    for c in range(NCORES):
        out[c * SH : (c + 1) * SH, :] = r.results[c]["out2T"].T
    return out, r


def kernel(x, adj, W1, b1, W2, b2):
    out, _ = _run(dict(x=x, adj=adj, W1=W1, b1=b1, W2=W2, b2=b2))
    return out
